# revision 1
# baseline (speedup 1.0000x reference)
"""LIF spiking-neuron scan (SimpleSNN) Trainium2 Bass kernel.

Reference semantics (per sample b, neuron n, over T timesteps):
    mem = mem * 0.9 + x[t]
    spike[t] = (mem >= 1.5)
    mem = mem * (1 - spike[t])

Full inputs [256, 200, 1024] f32 are sharded batch-wise over 8 NeuronCores
(32 samples/core; the time recurrence is per-sample so no cross-core comms).

Host-side, each core's shard [32, 200, 1024] is permuted to a
partition-major layout [128, 200, 256] with partition p = k*32 + b
(k = n // 256, b = sample), so every chunk DMA is a single dense 3-D
transfer carrying one completion semaphore.

Per-core device design:
  - The recurrence is rewritten over the PRE-reset membrane w:
        w_t = select(w_{t-1} < 1.5, w_{t-1}, 0) * 0.9 + x_t
        spike_t = (w_t >= 1.5)
    which is bit-identical to the reference (same two f32 roundings per
    step) and needs only ONE fused custom-DVE op per step (5 ALU stages
    of the DVE's 8-stage pipeline). The w history is materialized in the
    chunk tile, so the whole sequential chain is 200 back-to-back Vector
    engine instructions at ~[128, 256] each.
  - T=200 steps split into chunks of 25. Per chunk: one DMA load of
    x [128, 25, 256], 25 fused LIF-step ops (DVE), one batched GpSimd
    tensor_scalar over the w chunk (spikes = (w >= 1.5) as 1.0/0.0),
    one DMA store of the spike chunk. All DMAs are SWDGE (gpsimd).
  - Bacc lowering splits multi-wait instructions into event-semaphore
    chains (TRN2 allows at most one sync wait per instruction).
"""

from contextlib import ExitStack

import numpy as np

B, T, N = 256, 200, 1024
NCORES = 8
BL = B // NCORES  # 32 samples per core
DECAY = 0.9
TH = 1.5
P128 = 128
FREE = 256  # free-dim size of the state tile
NK = N // FREE  # 4 n-blocks; partition p = k*32 + b
# Ragged chunking: small first chunk (faster pipeline ramp — compute can
# start sooner) and small last chunk (faster tail — final spike pass and
# store cover fewer steps).
CHUNKS = [13] + [25] * 7 + [12]
TCMAX = max(CHUNKS)

_CACHE = {}

_LIF_OP_NAME = "LIF_STEP_ANT"


def _lif_reference(in0, in1, s0, s1, imm2):
    return (
        np.where(in0 < np.float32(s0), in0, np.float32(0.0)) * np.float32(s1) + in1
    ).astype(np.float32)


def _register_lif_op():
    """Register the fused LIF-step custom DVE op:
        out = select(in0 < s0, in0, 0) * s1 + in1
    (in0 = previous membrane w, in1 = x_t, s0 = threshold, s1 = decay).
    Registration is the runtime equivalent of appending to dve_ops.OPS;
    uops_sha is computed from the same lower() used at compile time.
    """
    import concourse.dve_ops as dve_ops
    from concourse.dve_ops import DveOp
    from concourse.dve_spec import C0, C1, Spec, Src0, Src1, Zero, lower, select
    from concourse.dve_uop import DveOpSpec

    if _LIF_OP_NAME in dve_ops._SUB_OPCODE_FOR_NAME:
        for op in dve_ops.OPS:
            if op.name == _LIF_OP_NAME:
                return op
        raise RuntimeError("LIF op registered but not in OPS")

    body = select(Src0 < C0, Src0, Zero) * C1 + Src1
    spec = Spec(body=body, reference=_lif_reference)
    row = dve_ops._CUSTOM_DVE_ROW_BASE + len(dve_ops.OPS)
    shas = {}
    for ver in ("v3", "v4"):
        uops = lower(spec, ver=ver)
        shas[ver] = DveOpSpec(
            name=_LIF_OP_NAME, opcode=row, uops=uops, rd1_en=True
        ).sha(ver)
    op = DveOp(_LIF_OP_NAME, spec, subdim=False, uops_sha=shas)
    dve_ops.OPS.append(op)
    dve_ops._SUB_OPCODE_FOR_NAME[_LIF_OP_NAME] = row
    dve_ops.CUSTOM_DVE_SPECS[_LIF_OP_NAME] = spec
    return op


def _build_bass(reps: int = 1):
    # reps > 1 repeats the whole pipeline on the same buffers (benchmarking
    # only — amortizes host dispatch overhead to expose the device time).
    import concourse.bacc as bacc
    import concourse.tile as tile
    from concourse import mybir

    lif_op = _register_lif_op()

    nc = bacc.Bacc(
        "TRN2",
        target_bir_lowering=False,
        debug=False,
        enable_asserts=False,
    )

    P = P128
    f32 = mybir.dt.float32
    op = mybir.AluOpType

    x_d = nc.dram_tensor("x", [P, T, FREE], f32, kind="ExternalInput").ap()
    s_d = nc.dram_tensor("spk", [P, T, FREE], f32, kind="ExternalOutput").ap()

    with ExitStack() as ctx:
        tc = ctx.enter_context(tile.TileContext(nc))
        xp = ctx.enter_context(tc.tile_pool(name="xp", bufs=2))
        wp = ctx.enter_context(tc.tile_pool(name="wp", bufs=2))
        sp = ctx.enter_context(tc.tile_pool(name="sp", bufs=2))
        st = ctx.enter_context(tc.tile_pool(name="st", bufs=1))

        zero = st.tile([P, FREE], f32)
        nc.vector.memset(zero[:], 0.0)

        wt_prev = None
        prev_tc = None
        for c, tcsz in enumerate(CHUNKS * reps):
            t0 = sum(CHUNKS[: c % len(CHUNKS)])
            xt = xp.tile([P, TCMAX, FREE], f32, tag="x")
            # Loads ride the SP HWDGE ring, stores the ACT HWDGE ring —
            # two independent DMA queues that overlap.
            nc.sync.dma_start(out=xt[:, :tcsz, :], in_=x_d[:, t0 : t0 + tcsz, :])

            wt = wp.tile([P, TCMAX, FREE], f32, tag="w")
            for j in range(tcsz):
                if c == 0 and j == 0:
                    w_in = zero[:]
                elif j == 0:
                    w_in = wt_prev[:, prev_tc - 1, :]
                else:
                    w_in = wt[:, j - 1, :]
                # w_t = select(w_{t-1} < TH, w_{t-1}, 0) * DECAY + x_t
                nc.vector._custom_dve(
                    lif_op,
                    out=wt[:, j, :],
                    in0=w_in,
                    in1=xt[:, j, :],
                    s0=TH,
                    s1=DECAY,
                )
            wt_prev = wt
            prev_tc = tcsz

            spt = sp.tile([P, TCMAX, FREE], f32, tag="s")
            # spikes = (w >= TH) as 1.0/0.0, batched over the whole chunk
            nc.gpsimd.tensor_scalar(
                out=spt[:, :tcsz, :].rearrange("p t f -> p (t f)"),
                in0=wt[:, :tcsz, :].rearrange("p t f -> p (t f)"),
                scalar1=TH,
                scalar2=None,
                op0=op.is_ge,
            )
            nc.scalar.dma_start(out=s_d[:, t0 : t0 + tcsz, :], in_=spt[:, :tcsz, :])

    # Bacc lowering: splits multi-wait instructions into event-semaphore
    # chains (TRN2 allows at most one sync wait per instruction), register
    # allocation, DCE.
    nc.compile()
    return nc


def _get_nc():
    if "nc" not in _CACHE:
        _CACHE["nc"] = _build_bass()
    return _CACHE["nc"]


def _shard_input(inputs: np.ndarray, i: int) -> np.ndarray:
    # [32, 200, 1024] -> [32, 200, 4, 256] -> [4, 32, 200, 256] -> [128, 200, 256]
    xi = inputs[i * BL : (i + 1) * BL]
    xi = xi.reshape(BL, T, NK, FREE).transpose(2, 0, 1, 3)
    return np.ascontiguousarray(xi).reshape(P128, T, FREE)


def _unshard_output(spk: np.ndarray) -> np.ndarray:
    # [128, 200, 256] -> [4, 32, 200, 256] -> [32, 200, 4, 256] -> [32, 200, 1024]
    s = spk.reshape(NK, BL, T, FREE).transpose(1, 2, 0, 3)
    return np.ascontiguousarray(s).reshape(BL, T, N)


def kernel(inputs: np.ndarray, trace: bool = False) -> np.ndarray:
    from concourse.bass_utils import run_bass_kernel_spmd

    inputs = np.ascontiguousarray(np.asarray(inputs, dtype=np.float32))
    assert inputs.shape == (B, T, N), inputs.shape

    nc = _get_nc()
    in_maps = [{"x": _shard_input(inputs, i)} for i in range(NCORES)]
    res = run_bass_kernel_spmd(
        nc, in_maps, core_ids=list(range(NCORES)), trace=trace
    )
    _CACHE["last_results"] = res
    out = np.concatenate(
        [_unshard_output(r["spk"]) for r in res.results], axis=0
    )
    return out



# revision 4
# speedup vs baseline: 6.8487x; 6.8487x over previous
"""LIF spiking-neuron scan (SimpleSNN) Trainium2 Bass kernel.

Reference semantics (per sample b, neuron n, over T timesteps):
    mem = mem * 0.9 + x[t]
    spike[t] = (mem >= 1.5)
    mem = mem * (1 - spike[t])

Full inputs [256, 200, 1024] f32 are sharded batch-wise over 8 NeuronCores
(32 samples/core; the time recurrence is per-sample so no cross-core comms).

Host-side, each core's shard [32, 200, 1024] is permuted to a
partition-major layout [128, 200, 256] with partition p = k*32 + b
(k = n // 256, b = sample), so every chunk DMA is a single dense 3-D
transfer carrying one completion semaphore.

Per-core device design:
  - The recurrence is rewritten over the PRE-reset membrane w:
        w_t = select(w_{t-1} < 1.5, w_{t-1}, 0) * 0.9 + x_t
        spike_t = (w_t >= 1.5)
    which is bit-identical to the reference (same two f32 roundings per
    step) and needs only ONE fused custom-DVE op per step (5 ALU stages
    of the DVE's 8-stage pipeline). The w history is materialized in the
    chunk tile, so the whole sequential chain is 200 back-to-back Vector
    engine instructions at ~[128, 256] each (~425 ns/op steady state).
  - T=200 steps split into chunks. Per chunk: one DMA load of
    x [128, tc, 256] (SP HWDGE ring), tc fused LIF-step ops (DVE), one
    Activation-engine sweep over the w chunk that emits uint8 spikes:
        spike_u8 = u8(Sigmoid(2^29 * w - 1.5*2^29))
    (1 ulp of w at 1.5 maps to +-64 -> sigmoid fully saturated to exact
    1.0/0.0; only w == 1.5 exactly, measure-zero, could differ), then
    one DMA store of the u8 chunk (ACT HWDGE ring). The uint8 store
    cuts store traffic 4x; the host converts spikes back to f32
    (exact 0.0/1.0) during unsharding.
  - The DVE chain (~85 us), ACT sweep (~53 us) and DMA (~84 us of
    traffic at ~390 GB/s) all run on different engines/queues and
    overlap; GpSimd (measured ~8.5 G elem/s, 18x below the DVE) is not
    used for compute at all.
"""

from contextlib import ExitStack

import numpy as np

B, T, N = 256, 200, 1024
NCORES = 8
BL = B // NCORES  # 32 samples per core
DECAY = 0.9
TH = 1.5
P128 = 128
FREE = 256  # free-dim size of the state tile
NK = N // FREE  # 4 n-blocks; partition p = k*32 + b
# Ragged chunking: small first chunk (faster pipeline ramp — compute can
# start sooner) and small last chunk (faster tail — final spike pass and
# store cover fewer steps).
CHUNKS = [13] + [25] * 7 + [12]
TCMAX = max(CHUNKS)
# Sigmoid threshold scale: 1.5 * 2^29 is exactly representable in f32 and
# one f32 ulp of w at 1.5 (1.19e-7) maps to +-64 — deep in sigmoid
# saturation, so the u8 output is an exact (w >= 1.5) indicator.
SIG_SCALE = float(2**29)
SIG_BIAS = -TH * SIG_SCALE
# Spike-sweep engine: "act_sigmoid" (Activation engine, frees the DVE for
# the serial chain) or "dve_isge" (native DVE tensor_scalar fallback).
SWEEP = "act_sigmoid"

_CACHE = {}

_LIF_OP_NAME = "LIF_STEP_ANT"


def _lif_reference(in0, in1, s0, s1, imm2):
    return (
        np.where(in0 < np.float32(s0), in0, np.float32(0.0)) * np.float32(s1) + in1
    ).astype(np.float32)


def _register_lif_op():
    """Register the fused LIF-step custom DVE op:
        out = select(in0 < s0, in0, 0) * s1 + in1
    (in0 = previous membrane w, in1 = x_t, s0 = threshold, s1 = decay).
    Registration is the runtime equivalent of appending to dve_ops.OPS;
    uops_sha is computed from the same lower() used at compile time.
    """
    import concourse.dve_ops as dve_ops
    from concourse.dve_ops import DveOp
    from concourse.dve_spec import C0, C1, Spec, Src0, Src1, Zero, lower, select
    from concourse.dve_uop import DveOpSpec

    if _LIF_OP_NAME in dve_ops._SUB_OPCODE_FOR_NAME:
        for op in dve_ops.OPS:
            if op.name == _LIF_OP_NAME:
                return op
        raise RuntimeError("LIF op registered but not in OPS")

    body = select(Src0 < C0, Src0, Zero) * C1 + Src1
    spec = Spec(body=body, reference=_lif_reference)
    row = dve_ops._CUSTOM_DVE_ROW_BASE + len(dve_ops.OPS)
    shas = {}
    for ver in ("v3", "v4"):
        uops = lower(spec, ver=ver)
        shas[ver] = DveOpSpec(
            name=_LIF_OP_NAME, opcode=row, uops=uops, rd1_en=True
        ).sha(ver)
    op = DveOp(_LIF_OP_NAME, spec, subdim=False, uops_sha=shas)
    dve_ops.OPS.append(op)
    dve_ops._SUB_OPCODE_FOR_NAME[_LIF_OP_NAME] = row
    dve_ops.CUSTOM_DVE_SPECS[_LIF_OP_NAME] = spec
    return op


def _build_bass(reps: int = 1):
    # reps > 1 repeats the whole pipeline on the same buffers (benchmarking
    # only — amortizes host dispatch overhead to expose the device time).
    import concourse.bacc as bacc
    import concourse.tile as tile
    from concourse import mybir

    lif_op = _register_lif_op()

    nc = bacc.Bacc(
        "TRN2",
        target_bir_lowering=False,
        debug=False,
        enable_asserts=False,
    )

    P = P128
    f32 = mybir.dt.float32
    u8 = mybir.dt.uint8
    op = mybir.AluOpType

    x_d = nc.dram_tensor("x", [P, T, FREE], f32, kind="ExternalInput").ap()
    s_d = nc.dram_tensor("spk", [P, T, FREE], u8, kind="ExternalOutput").ap()

    with ExitStack() as ctx:
        tc = ctx.enter_context(tile.TileContext(nc))
        xp = ctx.enter_context(tc.tile_pool(name="xp", bufs=2))
        wp = ctx.enter_context(tc.tile_pool(name="wp", bufs=2))
        sp = ctx.enter_context(tc.tile_pool(name="sp", bufs=2))
        st = ctx.enter_context(tc.tile_pool(name="st", bufs=1))

        zero = st.tile([P, FREE], f32)
        nc.vector.memset(zero[:], 0.0)
        sig_bias = st.tile([P, 1], f32)
        nc.vector.memset(sig_bias[:], SIG_BIAS)

        wt_prev = None
        prev_tc = None
        for c, tcsz in enumerate(CHUNKS * reps):
            t0 = sum(CHUNKS[: c % len(CHUNKS)])
            xt = xp.tile([P, TCMAX, FREE], f32, tag="x")
            # Loads ride the SP HWDGE ring, stores the ACT HWDGE ring —
            # two independent DMA queues that overlap.
            nc.sync.dma_start(out=xt[:, :tcsz, :], in_=x_d[:, t0 : t0 + tcsz, :])

            wt = wp.tile([P, TCMAX, FREE], f32, tag="w")
            for j in range(tcsz):
                if c == 0 and j == 0:
                    w_in = zero[:]
                elif j == 0:
                    w_in = wt_prev[:, prev_tc - 1, :]
                else:
                    w_in = wt[:, j - 1, :]
                # w_t = select(w_{t-1} < TH, w_{t-1}, 0) * DECAY + x_t
                nc.vector._custom_dve(
                    lif_op,
                    out=wt[:, j, :],
                    in0=w_in,
                    in1=xt[:, j, :],
                    s0=TH,
                    s1=DECAY,
                )
            wt_prev = wt
            prev_tc = tcsz

            spt = sp.tile([P, TCMAX, FREE], u8, tag="s")
            if SWEEP == "act_sigmoid":
                # spikes = u8(sigmoid(2^29*(w - 1.5))) — exact 0/1 indicator
                # of (w >= 1.5) up to the measure-zero w == 1.5 case.
                nc.scalar.activation(
                    out=spt[:, :tcsz, :].rearrange("p t f -> p (t f)"),
                    in_=wt[:, :tcsz, :].rearrange("p t f -> p (t f)"),
                    func=mybir.ActivationFunctionType.Sigmoid,
                    bias=sig_bias[:],
                    scale=SIG_SCALE,
                )
            else:
                nc.vector.tensor_scalar(
                    out=spt[:, :tcsz, :].rearrange("p t f -> p (t f)"),
                    in0=wt[:, :tcsz, :].rearrange("p t f -> p (t f)"),
                    scalar1=TH,
                    scalar2=None,
                    op0=op.is_ge,
                )
            nc.scalar.dma_start(out=s_d[:, t0 : t0 + tcsz, :], in_=spt[:, :tcsz, :])

    # Bacc lowering: splits multi-wait instructions into event-semaphore
    # chains (TRN2 allows at most one sync wait per instruction), register
    # allocation, DCE.
    nc.compile()
    return nc


def _get_nc():
    if "nc" not in _CACHE:
        _CACHE["nc"] = _build_bass()
    return _CACHE["nc"]


def _shard_input(inputs: np.ndarray, i: int) -> np.ndarray:
    # [32, 200, 1024] -> [32, 200, 4, 256] -> [4, 32, 200, 256] -> [128, 200, 256]
    xi = inputs[i * BL : (i + 1) * BL]
    xi = xi.reshape(BL, T, NK, FREE).transpose(2, 0, 1, 3)
    return np.ascontiguousarray(xi).reshape(P128, T, FREE)


def _unshard_output(spk: np.ndarray) -> np.ndarray:
    # u8 [128, 200, 256] -> [4, 32, 200, 256] -> [32, 200, 4, 256]
    # -> [32, 200, 1024] f32 (u8 spikes are exact 0/1)
    s = spk.reshape(NK, BL, T, FREE).transpose(1, 2, 0, 3)
    return np.ascontiguousarray(s).reshape(BL, T, N).astype(np.float32)


def kernel(inputs: np.ndarray, trace: bool = False) -> np.ndarray:
    from concourse.bass_utils import run_bass_kernel_spmd

    inputs = np.ascontiguousarray(np.asarray(inputs, dtype=np.float32))
    assert inputs.shape == (B, T, N), inputs.shape

    nc = _get_nc()
    in_maps = [{"x": _shard_input(inputs, i)} for i in range(NCORES)]
    res = run_bass_kernel_spmd(
        nc, in_maps, core_ids=list(range(NCORES)), trace=trace
    )
    _CACHE["last_results"] = res
    out = np.concatenate(
        [_unshard_output(r["spk"]) for r in res.results], axis=0
    )
    return out


# revision 8
# speedup vs baseline: 7.1203x; 1.0397x over previous
"""LIF spiking-neuron scan (SimpleSNN) Trainium2 Bass kernel.

Reference semantics (per sample b, neuron n, over T timesteps):
    mem = mem * 0.9 + x[t]
    spike[t] = (mem >= 1.5)
    mem = mem * (1 - spike[t])

Full inputs [256, 200, 1024] f32 are sharded batch-wise over 8 NeuronCores
(32 samples/core; the time recurrence is per-sample so no cross-core comms).

Host-side, each core's shard [32, 200, 1024] is permuted to a
partition-major layout [128, 200, 256] with partition p = k*32 + b
(k = n // 256, b = sample), so every chunk DMA is a single dense 3-D
transfer carrying one completion semaphore.

Per-core device design:
  - The recurrence is rewritten over the PRE-reset membrane w:
        w_t = select(w_{t-1} < 1.5, w_{t-1}, 0) * 0.9 + x_t
        spike_t = (w_t >= 1.5)
    which is bit-identical to the reference (same two f32 roundings per
    step) and needs only ONE fused custom-DVE op per step (5 ALU stages
    of the DVE's 8-stage pipeline). The w history is materialized in the
    chunk tile, so the whole sequential chain is 200 back-to-back Vector
    engine instructions at ~[128, 256] each (~425 ns/op steady state).
  - T=200 steps split into chunks. Per chunk: one DMA load of
    x [128, tc, 256] (SP HWDGE ring), tc fused LIF-step ops (DVE), one
    Activation-engine sweep over the w chunk that emits uint8 spikes:
        spike_u8 = u8(Sigmoid(2^29 * w - 1.5*2^29))
    (1 ulp of w at 1.5 maps to +-64 -> sigmoid fully saturated to exact
    1.0/0.0; only w == 1.5 exactly, measure-zero, could differ), then
    one DMA store of the u8 chunk (ACT HWDGE ring). The uint8 store
    cuts store traffic 4x; the host converts spikes back to f32
    (exact 0.0/1.0) during unsharding.
  - The DVE chain (~85 us), ACT sweep (~53 us) and DMA (~84 us of
    traffic at ~390 GB/s) all run on different engines/queues and
    overlap; GpSimd (measured ~8.5 G elem/s, 18x below the DVE) is not
    used for compute at all.
"""

from contextlib import ExitStack

import numpy as np

B, T, N = 256, 200, 1024
NCORES = 8
BL = B // NCORES  # 32 samples per core
DECAY = 0.9
TH = 1.5
P128 = 128
FREE = 256  # free-dim size of the state tile
NK = N // FREE  # 4 n-blocks; partition p = k*32 + b
# Ragged chunking: small growing head chunks (fast pipeline ramp — the
# first LIF op only waits on a 6-step load) and small tail chunks (the
# final sigmoid sweep + store cover few steps).
CHUNKS = [6, 10, 14, 20, 25, 25, 25, 25, 25, 15, 10]
assert sum(CHUNKS) == T
TCMAX = max(CHUNKS)
# Sigmoid threshold scale: 1.5 * 2^29 is exactly representable in f32 and
# one f32 ulp of w at 1.5 (1.19e-7) maps to +-64 — deep in sigmoid
# saturation, so the u8 output is an exact (w >= 1.5) indicator.
SIG_SCALE = float(2**29)
SIG_BIAS = -TH * SIG_SCALE
# Spike-sweep engine: "act_sigmoid" (Activation engine, frees the DVE for
# the serial chain) or "dve_isge" (native DVE tensor_scalar fallback).
SWEEP = "act_sigmoid"

_CACHE = {}

_LIF_OP_NAME = "LIF_STEP_ANT"


def _lif_reference(in0, in1, s0, s1, imm2):
    return (
        np.where(in0 < np.float32(s0), in0, np.float32(0.0)) * np.float32(s1) + in1
    ).astype(np.float32)


def _register_lif_op():
    """Register the fused LIF-step custom DVE op:
        out = select(in0 < s0, in0, 0) * s1 + in1
    (in0 = previous membrane w, in1 = x_t, s0 = threshold, s1 = decay).
    Registration is the runtime equivalent of appending to dve_ops.OPS;
    uops_sha is computed from the same lower() used at compile time.
    """
    import concourse.dve_ops as dve_ops
    from concourse.dve_ops import DveOp
    from concourse.dve_spec import C0, C1, Spec, Src0, Src1, Zero, lower, select
    from concourse.dve_uop import DveOpSpec

    if _LIF_OP_NAME in dve_ops._SUB_OPCODE_FOR_NAME:
        for op in dve_ops.OPS:
            if op.name == _LIF_OP_NAME:
                return op
        raise RuntimeError("LIF op registered but not in OPS")

    body = select(Src0 < C0, Src0, Zero) * C1 + Src1
    spec = Spec(body=body, reference=_lif_reference)
    row = dve_ops._CUSTOM_DVE_ROW_BASE + len(dve_ops.OPS)
    shas = {}
    for ver in ("v3", "v4"):
        uops = lower(spec, ver=ver)
        shas[ver] = DveOpSpec(
            name=_LIF_OP_NAME, opcode=row, uops=uops, rd1_en=True
        ).sha(ver)
    op = DveOp(_LIF_OP_NAME, spec, subdim=False, uops_sha=shas)
    dve_ops.OPS.append(op)
    dve_ops._SUB_OPCODE_FOR_NAME[_LIF_OP_NAME] = row
    dve_ops.CUSTOM_DVE_SPECS[_LIF_OP_NAME] = spec
    return op


def _build_bass(reps: int = 1):
    # reps > 1 repeats the whole pipeline on the same buffers (benchmarking
    # only — amortizes host dispatch overhead to expose the device time).
    import concourse.bacc as bacc
    import concourse.tile as tile
    from concourse import mybir

    lif_op = _register_lif_op()

    nc = bacc.Bacc(
        "TRN2",
        target_bir_lowering=False,
        debug=False,
        enable_asserts=False,
    )

    P = P128
    f32 = mybir.dt.float32
    u8 = mybir.dt.uint8
    op = mybir.AluOpType

    x_d = nc.dram_tensor("x", [P, T, FREE], f32, kind="ExternalInput").ap()
    s_d = nc.dram_tensor("spk", [P, T, FREE], u8, kind="ExternalOutput").ap()

    with ExitStack() as ctx:
        tc = ctx.enter_context(tile.TileContext(nc))
        xp = ctx.enter_context(tc.tile_pool(name="xp", bufs=3))
        wp = ctx.enter_context(tc.tile_pool(name="wp", bufs=2))
        sp = ctx.enter_context(tc.tile_pool(name="sp", bufs=2))
        st = ctx.enter_context(tc.tile_pool(name="st", bufs=1))

        zero = st.tile([P, FREE], f32)
        nc.vector.memset(zero[:], 0.0)
        sig_bias = st.tile([P, 1], f32)
        nc.vector.memset(sig_bias[:], SIG_BIAS)

        wt_prev = None
        prev_tc = None
        for c, tcsz in enumerate(CHUNKS * reps):
            t0 = sum(CHUNKS[: c % len(CHUNKS)])
            xt = xp.tile([P, TCMAX, FREE], f32, tag="x")
            # Loads alternate between the SP HWDGE ring and the GpSimd
            # SWDGE queue (two independent DMA paths, both otherwise idle)
            # so consecutive chunk loads overlap; stores ride the ACT ring.
            ld_eng = nc.sync if c % 2 == 0 else nc.gpsimd
            ld_eng.dma_start(out=xt[:, :tcsz, :], in_=x_d[:, t0 : t0 + tcsz, :])

            wt = wp.tile([P, TCMAX, FREE], f32, tag="w")
            for j in range(tcsz):
                if c == 0 and j == 0:
                    w_in = zero[:]
                elif j == 0:
                    w_in = wt_prev[:, prev_tc - 1, :]
                else:
                    w_in = wt[:, j - 1, :]
                # w_t = select(w_{t-1} < TH, w_{t-1}, 0) * DECAY + x_t
                nc.vector._custom_dve(
                    lif_op,
                    out=wt[:, j, :],
                    in0=w_in,
                    in1=xt[:, j, :],
                    s0=TH,
                    s1=DECAY,
                )
            wt_prev = wt
            prev_tc = tcsz

            spt = sp.tile([P, TCMAX, FREE], u8, tag="s")
            if SWEEP == "act_sigmoid":
                # spikes = u8(sigmoid(2^29*(w - 1.5))) — exact 0/1 indicator
                # of (w >= 1.5) up to the measure-zero w == 1.5 case.
                nc.scalar.activation(
                    out=spt[:, :tcsz, :].rearrange("p t f -> p (t f)"),
                    in_=wt[:, :tcsz, :].rearrange("p t f -> p (t f)"),
                    func=mybir.ActivationFunctionType.Sigmoid,
                    bias=sig_bias[:],
                    scale=SIG_SCALE,
                )
            else:
                nc.vector.tensor_scalar(
                    out=spt[:, :tcsz, :].rearrange("p t f -> p (t f)"),
                    in0=wt[:, :tcsz, :].rearrange("p t f -> p (t f)"),
                    scalar1=TH,
                    scalar2=None,
                    op0=op.is_ge,
                )
            nc.scalar.dma_start(out=s_d[:, t0 : t0 + tcsz, :], in_=spt[:, :tcsz, :])

    # Bacc lowering: splits multi-wait instructions into event-semaphore
    # chains (TRN2 allows at most one sync wait per instruction), register
    # allocation, DCE.
    nc.compile()
    return nc


def _get_nc():
    if "nc" not in _CACHE:
        _CACHE["nc"] = _build_bass()
    return _CACHE["nc"]


def _shard_input(inputs: np.ndarray, i: int) -> np.ndarray:
    # [32, 200, 1024] -> [32, 200, 4, 256] -> [4, 32, 200, 256] -> [128, 200, 256]
    xi = inputs[i * BL : (i + 1) * BL]
    xi = xi.reshape(BL, T, NK, FREE).transpose(2, 0, 1, 3)
    return np.ascontiguousarray(xi).reshape(P128, T, FREE)


def _unshard_output(spk: np.ndarray) -> np.ndarray:
    # u8 [128, 200, 256] -> [4, 32, 200, 256] -> [32, 200, 4, 256]
    # -> [32, 200, 1024] f32 (u8 spikes are exact 0/1)
    s = spk.reshape(NK, BL, T, FREE).transpose(1, 2, 0, 3)
    return np.ascontiguousarray(s).reshape(BL, T, N).astype(np.float32)


def kernel(inputs: np.ndarray, trace: bool = False) -> np.ndarray:
    from concourse.bass_utils import run_bass_kernel_spmd

    inputs = np.ascontiguousarray(np.asarray(inputs, dtype=np.float32))
    assert inputs.shape == (B, T, N), inputs.shape

    nc = _get_nc()
    in_maps = [{"x": _shard_input(inputs, i)} for i in range(NCORES)]
    res = run_bass_kernel_spmd(
        nc, in_maps, core_ids=list(range(NCORES)), trace=trace
    )
    _CACHE["last_results"] = res
    out = np.concatenate(
        [_unshard_output(r["spk"]) for r in res.results], axis=0
    )
    return out
